# revision 6
# baseline (speedup 1.0000x reference)
"""DRAW-style read attention on Trainium2 — data-parallel over batch on 8 NeuronCores.

reference math (per batch element):
    params = h @ W.T + b                         [5]
    g_x = 64.5*(p0+1)-0.5 ; g_y likewise
    sigma2 = exp(p2) ; delta = (127/31)*exp(p3) ; gamma = exp(p4)
    mu_x[n] = g_x + (n-15.5)*delta ; mu_y likewise
    FX[n,h] = exp(-(h-mu_x[n])^2/(2 sigma2)) / (Z_n + 1e-8)    (Z_n = row sum)
    FY[m,w] likewise
    patch_i = FX @ img_i @ FY.T   for img in (x_c0..2, xhat_c0..2)
    out = gamma * flatten(patches)               [6144]

device layout per core (local batch B=32):
    params via 8 accumulated K=128 matmuls; tiny transforms on DVE/ACT
    expand per-b scalars to the (b,n)-partition layout with 0/1 selection matmuls
    filterbanks built in [bn, hw] layout (free-axis normalize), gamma folded into FY,
    then PE-transposed into FXT/FYT [hw, bn] bf16 for use as matmul rhs
    main loop over b: At[w,n] = img[h,w].T @ FXT_b ; patch[n,m] = At.T @ FYT_b
    (FX normalizer folded into FXT, FY normalizer*gamma folded into FYT, so patch
    in PSUM is final) -> ACT copy -> DMA to out rows
"""

import math

import numpy as np

import concourse.bass as bass  # noqa: F401  (import keeps bass registered)
import concourse.mybir as mybir
import concourse.tile as tile
from concourse import bacc
from concourse.bass_utils import run_bass_kernel_spmd
from concourse.masks import make_identity

F32 = mybir.dt.float32
BF16 = mybir.dt.bfloat16

NCORES = 8
B = 32          # per-core batch shard
C = 3
IMG = 128
N = 32
DH = 1024
U = 2 * C       # images per batch element: x channels 0..2 then x_hat channels 0..2
NT = (B * N) // 128   # tiles over the flattened (b, n) axis
KC = DH // 128        # contraction chunks for the params matmul
DELTA_NORM = (IMG - 1.0) / (N - 1.0)
EPS = 1e-8


def build_nc(finalize=True):
    nc = bacc.Bacc("TRN2", target_bir_lowering=False, debug=False, num_devices=NCORES)
    AFT = mybir.ActivationFunctionType
    ALU = mybir.AluOpType

    x_d = nc.declare_dram_parameter("x", [B, C, IMG, IMG], F32, isOutput=False)
    xh_d = nc.declare_dram_parameter("xh", [B, C, IMG, IMG], F32, isOutput=False)
    hT_d = nc.declare_dram_parameter("hT", [DH, B], F32, isOutput=False)
    wT_d = nc.declare_dram_parameter("wT", [DH, 5], F32, isOutput=False)
    bias_d = nc.declare_dram_parameter("bias", [B, 5], F32, isOutput=False)
    E_d = nc.declare_dram_parameter("E", [NT, B, 128], F32, isOutput=False)
    offs_d = nc.declare_dram_parameter("offs", [128, 1], F32, isOutput=False)
    grid_d = nc.declare_dram_parameter("grid", [128, IMG], F32, isOutput=False)
    out_d = nc.declare_dram_parameter("out", [B, U * N * N], F32, isOutput=True)

    with tile.TileContext(nc) as tc:
        with (
            tc.tile_pool(name="consts", bufs=1) as consts,
            tc.tile_pool(name="fb", bufs=4) as fb,
            tc.tile_pool(name="imgf_p", bufs=3) as imgf_p,
            tc.tile_pool(name="imgb_p", bufs=3) as imgb_p,
            tc.tile_pool(name="atb_p", bufs=3) as atb_p,
            tc.tile_pool(name="outs_p", bufs=3) as outs_p,
            tc.tile_pool(name="ps_pro", bufs=1, space="PSUM") as ps_pro,
            tc.tile_pool(name="ps_tr", bufs=2, space="PSUM") as ps_tr,
            tc.tile_pool(name="ps_at", bufs=2, space="PSUM") as ps_at,
            tc.tile_pool(name="ps_pt", bufs=2, space="PSUM") as ps_pt,
        ):
            # ---- constants / small inputs ----
            hT_sb = consts.tile([128, KC, B], F32)
            nc.sync.dma_start(out=hT_sb, in_=hT_d[:].rearrange("(k p) b -> p k b", p=128))
            wT_sb = consts.tile([128, KC, 5], F32)
            nc.sync.dma_start(out=wT_sb, in_=wT_d[:].rearrange("(k p) j -> p k j", p=128))
            bias_sb = consts.tile([B, 5], F32)
            nc.sync.dma_start(out=bias_sb, in_=bias_d[:])
            E_sb = consts.tile([B, NT, 128], F32)
            nc.sync.dma_start(out=E_sb, in_=E_d[:].rearrange("t b p -> b t p"))
            offs_sb = consts.tile([128, 1], F32)
            nc.sync.dma_start(out=offs_sb, in_=offs_d[:])
            grid_sb = consts.tile([128, IMG], F32)
            nc.sync.dma_start(out=grid_sb, in_=grid_d[:])
            ident = consts.tile([128, 128], BF16)
            make_identity(nc, ident)
            zeros = consts.tile([128, 1], F32)
            nc.vector.memset(zeros, 0.0)

            # ---- params = h @ W.T + b  -> [B, 5] ----
            ps_par = ps_pro.tile([B, 5], F32)
            for k in range(KC):
                nc.tensor.matmul(ps_par, hT_sb[:, k, :], wT_sb[:, k, :],
                                 start=(k == 0), stop=(k == KC - 1))
            tp = consts.tile([B, 5], F32)
            nc.vector.tensor_add(tp, ps_par, bias_sb)

            # ---- transforms -> tp2 cols = [g_x, g_y, -1/(2*sigma2), delta, gamma] ----
            tp2 = consts.tile([B, 5], F32)
            half = (IMG + 1) / 2.0
            nc.vector.tensor_scalar(tp2[:, 0:1], tp[:, 0:1], half, half - 0.5,
                                    op0=ALU.mult, op1=ALU.add)
            nc.vector.tensor_scalar(tp2[:, 1:2], tp[:, 1:2], half, half - 0.5,
                                    op0=ALU.mult, op1=ALU.add)
            tmp_is = consts.tile([B, 1], F32)
            nc.scalar.activation(tmp_is, tp[:, 2:3], AFT.Exp, scale=-1.0,
                                 bias=zeros[:B])
            nc.vector.tensor_scalar_mul(tp2[:, 2:3], tmp_is, -0.5)
            tmp_d = consts.tile([B, 1], F32)
            nc.scalar.activation(tmp_d, tp[:, 3:4], AFT.Exp, bias=zeros[:B])
            nc.vector.tensor_scalar_mul(tp2[:, 3:4], tmp_d, DELTA_NORM)
            nc.scalar.activation(tp2[:, 4:5], tp[:, 4:5], AFT.Exp, bias=zeros[:B])

            # ---- expand per-b scalars to (b,n) partitions: ep [128, NT, 5] ----
            ps_e = ps_pro.tile([128, NT, 5], F32)
            for t in range(NT):
                nc.tensor.matmul(ps_e[:, t, :], E_sb[:, t, :], tp2, start=True, stop=True)
            ep = consts.tile([128, NT, 5], F32)
            nc.vector.tensor_copy(ep, ps_e)

            mu_x = consts.tile([128, NT], F32)
            nc.vector.scalar_tensor_tensor(mu_x, ep[:, :, 3], offs_sb, ep[:, :, 0],
                                           op0=ALU.mult, op1=ALU.add)
            mu_y = consts.tile([128, NT], F32)
            nc.vector.scalar_tensor_tensor(mu_y, ep[:, :, 3], offs_sb, ep[:, :, 1],
                                           op0=ALU.mult, op1=ALU.add)

            # ---- filterbanks: build [bn, hw], normalize, transpose to [hw, bn] bf16 ----
            FXT = consts.tile([128, B * N], BF16)
            FYT = consts.tile([128, B * N], BF16)
            for t in range(NT):
                for mu, FT, fold_gamma in ((mu_x, FXT, False), (mu_y, FYT, True)):
                    d = fb.tile([128, IMG], F32, tag="d")
                    nc.vector.tensor_scalar_sub(d, grid_sb, mu[:, t:t + 1])
                    nc.vector.tensor_mul(d, d, d)
                    e_un = fb.tile([128, IMG], F32, tag="e_un")
                    nc.scalar.activation(e_un, d, AFT.Exp, scale=ep[:, t, 2:3],
                                         bias=zeros)
                    Z = fb.tile([128, 1], F32, tag="Z")
                    nc.vector.tensor_reduce(Z, e_un, axis=mybir.AxisListType.X,
                                            op=ALU.add)
                    nc.vector.tensor_scalar_add(Z, Z, EPS)
                    invZ = fb.tile([128, 1], F32, tag="invZ")
                    nc.vector.reciprocal(invZ, Z)
                    if fold_gamma:
                        nc.vector.tensor_mul(invZ, invZ, ep[:, t, 4:5])
                    Fn = fb.tile([128, IMG], BF16, tag="Fn")
                    nc.vector.tensor_scalar_mul(Fn, e_un, invZ)
                    ps_t = ps_tr.tile([128, 128], BF16)
                    nc.tensor.transpose(ps_t, Fn, ident)
                    nc.vector.tensor_copy(FT[:, t * 128:(t + 1) * 128], ps_t)

            # ---- main loop over local batch, software-pipelined ----
            out_v = out_d[:].rearrange("B (u n m) -> B n u m", u=U, n=N)

            def load_mm1(b):
                imgf = imgf_p.tile([128, U, IMG], F32, tag="imgf")
                nc.sync.dma_start(out=imgf[:, 0:C, :],
                                  in_=x_d[b].rearrange("c h w -> h c w"))
                nc.sync.dma_start(out=imgf[:, C:U, :],
                                  in_=xh_d[b].rearrange("c h w -> h c w"))
                imgb = imgb_p.tile([128, U, IMG], BF16, tag="imgb")
                nc.vector.tensor_copy(imgb, imgf)
                ps_a = ps_at.tile([128, U, N], F32)
                for u in range(U):
                    nc.tensor.matmul(ps_a[:, u, :], imgb[:, u, :],
                                     FXT[:, b * N:(b + 1) * N], start=True, stop=True)
                atb = atb_p.tile([128, U, N], BF16, tag="atb")
                nc.vector.tensor_copy(atb, ps_a)
                return atb

            def mm2_store(b, atb):
                ps_p = ps_pt.tile([N, U, N], F32)
                for u in range(U):
                    nc.tensor.matmul(ps_p[:, u, :], atb[:, u, :],
                                     FYT[:, b * N:(b + 1) * N], start=True, stop=True)
                outs = outs_p.tile([N, U, N], F32, tag="outs")
                nc.scalar.copy(outs, ps_p)
                nc.sync.dma_start(out=out_v[b], in_=outs)

            prev = None
            for b in range(B):
                atb = load_mm1(b)
                if prev is not None:
                    mm2_store(*prev)
                prev = (b, atb)
            mm2_store(*prev)

    if finalize:
        nc.finalize()
    return nc


_CACHE = {}


def _get_nc():
    if "nc" not in _CACHE:
        _CACHE["nc"] = build_nc()
    return _CACHE["nc"]


def host_constants():
    E = np.zeros((NT, B, 128), np.float32)
    for t in range(NT):
        for p in range(128):
            E[t, (t * 128 + p) // N, p] = 1.0
    offs = (np.arange(128) % N - (N / 2.0 - 0.5)).astype(np.float32).reshape(128, 1)
    grid = np.broadcast_to(np.arange(IMG, dtype=np.float32), (128, IMG)).copy()
    return E, offs, grid


def make_in_maps(x, x_hat, h_dec_prev, W_read, b_read):
    x = np.asarray(x, np.float32)
    x_hat = np.asarray(x_hat, np.float32)
    h = np.asarray(h_dec_prev, np.float32)
    E, offs, grid = host_constants()
    wT = np.ascontiguousarray(np.asarray(W_read, np.float32).T)
    bias = np.broadcast_to(np.asarray(b_read, np.float32), (B, 5)).copy()
    in_maps = []
    for i in range(NCORES):
        sl = slice(i * B, (i + 1) * B)
        in_maps.append({
            "x": np.ascontiguousarray(x[sl]),
            "xh": np.ascontiguousarray(x_hat[sl]),
            "hT": np.ascontiguousarray(h[sl].T),
            "wT": wT,
            "bias": bias,
            "E": E,
            "offs": offs,
            "grid": grid,
        })
    return in_maps


def _install_ntff_hook():
    """The container's antenv package lacks axon_hooks; provide it so
    run_bass_kernel_spmd(trace=True) can capture an NTFF profile."""
    import sys
    import types
    if "antenv.axon_hooks" in sys.modules:
        return
    try:
        from trn_agent_boot.trn_boot import _ntff_profile_via_ctypes
    except ImportError:
        return
    mod = types.ModuleType("antenv.axon_hooks")
    hook = [_ntff_profile_via_ctypes("/opt/axon/libaxon_pjrt.so")]
    mod.set_axon_ntff_profile_hook = lambda h: hook.__setitem__(0, h)
    mod.get_axon_ntff_profile_hook = lambda: hook[0]
    sys.modules["antenv.axon_hooks"] = mod
    try:
        import antenv
        antenv.axon_hooks = mod
    except ImportError:
        pass


def run(inputs, trace=False, **spmd_kwargs):
    """Run on the 8 NeuronCores; returns (out [256, 6144] f32, BassKernelResults)."""
    if trace:
        _install_ntff_hook()
    nc = _get_nc()
    in_maps = make_in_maps(**inputs)
    res = run_bass_kernel_spmd(nc, in_maps, core_ids=list(range(NCORES)),
                               trace=trace, **spmd_kwargs)
    out = np.concatenate([res.results[i]["out"] for i in range(NCORES)], axis=0)
    return out.astype(np.float32, copy=False), res


def kernel(x, x_hat, h_dec_prev, W_read, b_read):
    out, _ = run(dict(x=x, x_hat=x_hat, h_dec_prev=h_dec_prev,
                      W_read=W_read, b_read=b_read))
    return out


# revision 9
# speedup vs baseline: 1.1607x; 1.1607x over previous
"""DRAW-style read attention on Trainium2 — data-parallel over batch on 8 NeuronCores.

reference math (per batch element):
    params = h @ W.T + b                         [5]
    g_x = 64.5*(p0+1)-0.5 ; g_y likewise
    sigma2 = exp(p2) ; delta = (127/31)*exp(p3) ; gamma = exp(p4)
    mu_x[n] = g_x + (n-15.5)*delta ; mu_y likewise
    FX[n,h] = exp(-(h-mu_x[n])^2/(2 sigma2)) / (Z_n + 1e-8)    (Z_n = row sum)
    FY[m,w] likewise
    patch_i = FX @ img_i @ FY.T   for img in (x_c0..2, xhat_c0..2)
    out = gamma * flatten(patches)               [6144]

device layout per core (local batch B=32):
    params via 8 accumulated K=128 matmuls; tiny transforms on DVE/ACT
    expand per-b scalars to the (b,n)-partition layout with 0/1 selection matmuls
    filterbanks built in [bn, hw] layout (free-axis normalize), gamma folded into FY,
    then PE-transposed into FXT/FYT [hw, bn] bf16 for use as matmul rhs
    main loop over b: At[w,n] = img[h,w].T @ FXT_b ; patch[n,m] = At.T @ FYT_b
    (FX normalizer folded into FXT, FY normalizer*gamma folded into FYT, so patch
    in PSUM is final) -> ACT copy -> DMA to out rows
"""

import math

import numpy as np

import concourse.bass as bass  # noqa: F401  (import keeps bass registered)
import concourse.mybir as mybir
import concourse.tile as tile
from concourse import bacc
from concourse.bass_utils import run_bass_kernel_spmd
from concourse.masks import make_identity

F32 = mybir.dt.float32
BF16 = mybir.dt.bfloat16

NCORES = 8
B = 32          # per-core batch shard
C = 3
IMG = 128
N = 32
DH = 1024
U = 2 * C       # images per batch element: x channels 0..2 then x_hat channels 0..2
NT = (B * N) // 128   # tiles over the flattened (b, n) axis
KC = DH // 128        # contraction chunks for the params matmul
DELTA_NORM = (IMG - 1.0) / (N - 1.0)
EPS = 1e-8


def build_nc(finalize=True):
    nc = bacc.Bacc("TRN2", target_bir_lowering=False, debug=False, num_devices=NCORES)
    AFT = mybir.ActivationFunctionType
    ALU = mybir.AluOpType

    x_d = nc.declare_dram_parameter("x", [B, C, IMG, IMG], F32, isOutput=False)
    xh_d = nc.declare_dram_parameter("xh", [B, C, IMG, IMG], F32, isOutput=False)
    hT_d = nc.declare_dram_parameter("hT", [DH, B], F32, isOutput=False)
    wT_d = nc.declare_dram_parameter("wT", [DH, 5], F32, isOutput=False)
    bias_d = nc.declare_dram_parameter("bias", [B, 5], F32, isOutput=False)
    E_d = nc.declare_dram_parameter("E", [NT, B, 128], F32, isOutput=False)
    offs_d = nc.declare_dram_parameter("offs", [128, 1], F32, isOutput=False)
    grid_d = nc.declare_dram_parameter("grid", [128, IMG], F32, isOutput=False)
    out_d = nc.declare_dram_parameter("out", [B, U * N * N], F32, isOutput=True)

    with tile.TileContext(nc) as tc:
        with (
            tc.tile_pool(name="consts", bufs=1) as consts,
            tc.tile_pool(name="fb", bufs=4) as fb,
            tc.tile_pool(name="imgf_p", bufs=3) as imgf_p,
            tc.tile_pool(name="imgb_p", bufs=3) as imgb_p,
            tc.tile_pool(name="atb_p", bufs=3) as atb_p,
            tc.tile_pool(name="outs_p", bufs=3) as outs_p,
            tc.tile_pool(name="ps_pro", bufs=1, space="PSUM") as ps_pro,
            tc.tile_pool(name="ps_tr", bufs=2, space="PSUM") as ps_tr,
            tc.tile_pool(name="ps_at", bufs=2, space="PSUM") as ps_at,
            tc.tile_pool(name="ps_pt", bufs=2, space="PSUM") as ps_pt,
        ):
            # ---- constants / small inputs ----
            hT_sb = consts.tile([128, KC, B], F32)
            nc.sync.dma_start(out=hT_sb, in_=hT_d[:].rearrange("(k p) b -> p k b", p=128))
            wT_sb = consts.tile([128, KC, 5], F32)
            nc.sync.dma_start(out=wT_sb, in_=wT_d[:].rearrange("(k p) j -> p k j", p=128))
            bias_sb = consts.tile([B, 5], F32)
            nc.sync.dma_start(out=bias_sb, in_=bias_d[:])
            E_sb = consts.tile([B, NT, 128], F32)
            nc.sync.dma_start(out=E_sb, in_=E_d[:].rearrange("t b p -> b t p"))
            offs_sb = consts.tile([128, 1], F32)
            nc.sync.dma_start(out=offs_sb, in_=offs_d[:])
            grid_sb = consts.tile([128, IMG], F32)
            nc.sync.dma_start(out=grid_sb, in_=grid_d[:])
            ident = consts.tile([128, 128], BF16)
            make_identity(nc, ident)
            zeros = consts.tile([128, 1], F32)
            nc.vector.memset(zeros, 0.0)

            # ---- params = h @ W.T + b  -> [B, 5] ----
            ps_par = ps_pro.tile([B, 5], F32)
            for k in range(KC):
                nc.tensor.matmul(ps_par, hT_sb[:, k, :], wT_sb[:, k, :],
                                 start=(k == 0), stop=(k == KC - 1))
            tp = consts.tile([B, 5], F32)
            nc.vector.tensor_add(tp, ps_par, bias_sb)

            # ---- transforms -> tp2 cols = [g_x, g_y, s=sqrt(1/(2*sigma2)), delta, gamma] ----
            tp2 = consts.tile([B, 5], F32)
            half = (IMG + 1) / 2.0
            nc.vector.tensor_scalar(tp2[:, 0:1], tp[:, 0:1], half, half - 0.5,
                                    op0=ALU.mult, op1=ALU.add)
            nc.vector.tensor_scalar(tp2[:, 1:2], tp[:, 1:2], half, half - 0.5,
                                    op0=ALU.mult, op1=ALU.add)
            tmp_is = consts.tile([B, 1], F32)
            nc.scalar.activation(tmp_is, tp[:, 2:3], AFT.Exp, scale=-0.5,
                                 bias=zeros[:B])
            nc.vector.tensor_scalar_mul(tp2[:, 2:3], tmp_is, math.sqrt(0.5))
            tmp_d = consts.tile([B, 1], F32)
            nc.scalar.activation(tmp_d, tp[:, 3:4], AFT.Exp, bias=zeros[:B])
            nc.vector.tensor_scalar_mul(tp2[:, 3:4], tmp_d, DELTA_NORM)
            nc.scalar.activation(tp2[:, 4:5], tp[:, 4:5], AFT.Exp, bias=zeros[:B])

            # ---- expand per-b scalars to (b,n) partitions: ep [128, NT, 5] ----
            ps_e = ps_pro.tile([128, NT, 5], F32)
            for t in range(NT):
                nc.tensor.matmul(ps_e[:, t, :], E_sb[:, t, :], tp2, start=True, stop=True)
            ep = consts.tile([128, NT, 5], F32)
            nc.vector.tensor_copy(ep, ps_e)

            mu_x = consts.tile([128, NT], F32)
            nc.vector.scalar_tensor_tensor(mu_x, ep[:, :, 3], offs_sb, ep[:, :, 0],
                                           op0=ALU.mult, op1=ALU.add)
            mu_y = consts.tile([128, NT], F32)
            nc.vector.scalar_tensor_tensor(mu_y, ep[:, :, 3], offs_sb, ep[:, :, 1],
                                           op0=ALU.mult, op1=ALU.add)
            # bias terms for the Square trick: -mu*s
            nsmu_x = consts.tile([128, NT], F32)
            nc.vector.scalar_tensor_tensor(nsmu_x, mu_x, -1.0, ep[:, :, 2],
                                           op0=ALU.mult, op1=ALU.mult)
            nsmu_y = consts.tile([128, NT], F32)
            nc.vector.scalar_tensor_tensor(nsmu_y, mu_y, -1.0, ep[:, :, 2],
                                           op0=ALU.mult, op1=ALU.mult)

            FXT = consts.tile([128, B * N], BF16)
            FYT = consts.tile([128, B * N], BF16)

            def fbank(t, nsmu, FT, fold_gamma):
                # sq = (s*grid - s*mu)^2 = (grid-mu)^2/(2 sigma2)  (one ACT op)
                sq = fb.tile([128, IMG], F32, tag="sq")
                nc.scalar.activation(sq, grid_sb, AFT.Square,
                                     scale=ep[:, t, 2:3], bias=nsmu[:, t:t + 1])
                e_un = fb.tile([128, IMG], F32, tag="e_un")
                nc.scalar.activation(e_un, sq, AFT.Exp, scale=-1.0, bias=zeros)
                Z = fb.tile([128, 1], F32, tag="Z")
                nc.vector.tensor_reduce(Z, e_un, axis=mybir.AxisListType.X,
                                        op=ALU.add)
                nc.vector.tensor_scalar_add(Z, Z, EPS)
                invZ = fb.tile([128, 1], F32, tag="invZ")
                nc.vector.reciprocal(invZ, Z)
                if fold_gamma:
                    nc.vector.tensor_mul(invZ, invZ, ep[:, t, 4:5])
                Fn = fb.tile([128, IMG], BF16, tag="Fn")
                nc.vector.tensor_scalar_mul(Fn, e_un, invZ)
                ps_t = ps_tr.tile([128, 128], BF16)
                nc.tensor.transpose(ps_t, Fn, ident)
                nc.vector.tensor_copy(FT[:, t * 128:(t + 1) * 128], ps_t)

            # ---- main loop: pairs of batch elements, interleaved with the
            # filterbank tiles they depend on; mm2 pipelined one pair behind ----
            NP = B // 2
            out_v = out_d[:].rearrange("(P b2) (u n m) -> P n b2 u m",
                                       b2=2, u=U, n=N)

            def load_mm1(P):
                # img tiles laid out [h, i(x/xhat), b2, c, w] so each source
                # image block lands in one contiguous (mergeable) DMA dest
                imgf = imgf_p.tile([128, 2, 2, C, IMG], F32, tag="imgf")
                nc.sync.dma_start(out=imgf[:, 0],
                                  in_=x_d[2 * P:2 * P + 2].rearrange("b c h w -> h (b c) w"))
                nc.sync.dma_start(out=imgf[:, 1],
                                  in_=xh_d[2 * P:2 * P + 2].rearrange("b c h w -> h (b c) w"))
                imgb = imgb_p.tile([128, 2, 2, C, IMG], BF16, tag="imgb")
                # split the big f32->bf16 casts between GpSimd and DVE
                eng = nc.gpsimd if P % 2 == 0 else nc.vector
                eng.tensor_copy(imgb, imgf)
                ps_a = ps_at.tile([128, 2, U, N], F32)
                for b2 in range(2):
                    b = 2 * P + b2
                    for i in range(2):
                        for c in range(C):
                            nc.tensor.matmul(ps_a[:, b2, i * C + c, :],
                                             imgb[:, i, b2, c, :],
                                             FXT[:, b * N:(b + 1) * N],
                                             start=True, stop=True)
                atb = atb_p.tile([128, 2, U, N], BF16, tag="atb")
                nc.vector.tensor_copy(atb, ps_a)
                return atb

            def mm2_store(P, atb):
                ps_p = ps_pt.tile([N, 2, U, N], F32)
                for b2 in range(2):
                    b = 2 * P + b2
                    for u in range(U):
                        nc.tensor.matmul(ps_p[:, b2, u, :], atb[:, b2, u, :],
                                         FYT[:, b * N:(b + 1) * N],
                                         start=True, stop=True)
                outs = outs_p.tile([N, 2, U, N], F32, tag="outs")
                nc.any.tensor_copy(outs, ps_p)
                nc.sync.dma_start(out=out_v[P], in_=outs)

            prev = None
            for t in range(NT):
                fbank(t, nsmu_x, FXT, False)
                fbank(t, nsmu_y, FYT, True)
                for P in (2 * t, 2 * t + 1):
                    atb = load_mm1(P)
                    if prev is not None:
                        mm2_store(*prev)
                    prev = (P, atb)
            mm2_store(*prev)

    if finalize:
        nc.finalize()
    return nc


_CACHE = {}


def _get_nc():
    if "nc" not in _CACHE:
        _CACHE["nc"] = build_nc()
    return _CACHE["nc"]


def host_constants():
    E = np.zeros((NT, B, 128), np.float32)
    for t in range(NT):
        for p in range(128):
            E[t, (t * 128 + p) // N, p] = 1.0
    offs = (np.arange(128) % N - (N / 2.0 - 0.5)).astype(np.float32).reshape(128, 1)
    grid = np.broadcast_to(np.arange(IMG, dtype=np.float32), (128, IMG)).copy()
    return E, offs, grid


def make_in_maps(x, x_hat, h_dec_prev, W_read, b_read):
    x = np.asarray(x, np.float32)
    x_hat = np.asarray(x_hat, np.float32)
    h = np.asarray(h_dec_prev, np.float32)
    E, offs, grid = host_constants()
    wT = np.ascontiguousarray(np.asarray(W_read, np.float32).T)
    bias = np.broadcast_to(np.asarray(b_read, np.float32), (B, 5)).copy()
    in_maps = []
    for i in range(NCORES):
        sl = slice(i * B, (i + 1) * B)
        in_maps.append({
            "x": np.ascontiguousarray(x[sl]),
            "xh": np.ascontiguousarray(x_hat[sl]),
            "hT": np.ascontiguousarray(h[sl].T),
            "wT": wT,
            "bias": bias,
            "E": E,
            "offs": offs,
            "grid": grid,
        })
    return in_maps


def _install_ntff_hook():
    """The container's antenv package lacks axon_hooks; provide it so
    run_bass_kernel_spmd(trace=True) can capture an NTFF profile."""
    import sys
    import types
    if "antenv.axon_hooks" in sys.modules:
        return
    try:
        from trn_agent_boot.trn_boot import _ntff_profile_via_ctypes
    except ImportError:
        return
    mod = types.ModuleType("antenv.axon_hooks")
    hook = [_ntff_profile_via_ctypes("/opt/axon/libaxon_pjrt.so")]
    mod.set_axon_ntff_profile_hook = lambda h: hook.__setitem__(0, h)
    mod.get_axon_ntff_profile_hook = lambda: hook[0]
    sys.modules["antenv.axon_hooks"] = mod
    try:
        import antenv
        antenv.axon_hooks = mod
    except ImportError:
        pass


def run(inputs, trace=False, **spmd_kwargs):
    """Run on the 8 NeuronCores; returns (out [256, 6144] f32, BassKernelResults)."""
    if trace:
        _install_ntff_hook()
    nc = _get_nc()
    in_maps = make_in_maps(**inputs)
    res = run_bass_kernel_spmd(nc, in_maps, core_ids=list(range(NCORES)),
                               trace=trace, **spmd_kwargs)
    out = np.concatenate([res.results[i]["out"] for i in range(NCORES)], axis=0)
    return out.astype(np.float32, copy=False), res


def kernel(x, x_hat, h_dec_prev, W_read, b_read):
    out, _ = run(dict(x=x, x_hat=x_hat, h_dec_prev=h_dec_prev,
                      W_read=W_read, b_read=b_read))
    return out
